# revision 13
# baseline (speedup 1.0000x reference)
"""Trainium2 Bass kernel for nn_Attention_7679401525457.

score_i = relu(Linear(tanh(concat(h_i, z)))); alphas = softmax(scores);
attention = sum_i alphas_i * h_i.

Data-parallel over 8 NeuronCores: batch dim (32) sharded 4-per-core; the
tiny W/b replicated. Each core reads its encoder slice from HBM exactly
once in ~1 MiB chunks and computes behind the DMA stream.

Performance structure:
- The 16 big chunk DMAs own the SP HWDGE ring exclusively. HWDGE rings
  stall at the sequencer while the head DMA waits on its semaphore, so
  the tiny output-row DMAs (which wait on epilogue copies) go on the
  Activation ring and the constant loads go through the GPSIMD SWDGE
  ring -- the chunk stream never blocks.
- Chunk pipeline is emitted with a one-chunk lookahead: ACT's stream is
  tanh(k), tanh(k+1), exp(k), ... so exp (which needs DVE's score
  reduction of chunk k) doesn't stall ACT.
- Alphas stay unnormalized (relu keeps scores >= 0 so exp is bounded);
  the weighted sum accumulates on the PE incrementally and the softmax
  normalization folds into the final output-row scale. The denominator
  partition-sum rides on the PE (ones-vector matmul).
- The last batch streams its final two tiles as single-tile chunks and
  the epilogue copies run on ACT and DVE in parallel, shrinking the
  serial tail after the last DMA.
"""

import numpy as np

import concourse.bass as bass
import concourse.bacc as bacc
import concourse.mybir as mybir
import concourse.tile as tile
from concourse.bass_utils import run_bass_kernel_spmd

B, S, D = 32, 1024, 1024
NCORES = 8
BPC = B // NCORES  # batches per core
NT = S // 128  # s-tiles per batch
F32 = mybir.dt.float32
F32R = mybir.dt.float32r
BF16 = mybir.dt.bfloat16
AF = mybir.ActivationFunctionType
ALU = mybir.AluOpType

# float32r: same bits as fp32, PE matmul streams 1 col/cycle (vs 1/4 for
# strict fp32). Toggle if precision requires full fp32.
USE_F32R = True

# chunk plan per batch: list of (tile_start, n_tiles). Last batch ends
# with single-tile chunks to shorten the post-stream serial tail.
CHUNK4 = False  # 2 MiB chunks for all but the last batch
STAG = False    # staggered_reset on the timing-harness hardware loop
TTB = 3         # ttp pool bufs
LOOKA = 1       # exp lookahead distance in chunks
JB16 = False    # junk (STT dummy output) in bf16
EPI_ACT = False # both epilogue copies on ACT (else split ACT/DVE)
DUALRING = False  # alternate chunk DMAs across SP and ACT HWDGE rings
BODYX = 1       # bodies emitted per hardware-loop iteration (harness)


def _chunk_plan(b):
    if b < BPC - 1:
        if CHUNK4:
            return [(0, 4), (4, 4)]
        return [(0, 2), (2, 2), (4, 2), (6, 2)]
    return [(0, 2), (2, 2), (4, 2), (6, 1), (7, 1)]


def _chunks():
    return [(b, t0, nt) for b in range(BPC) for (t0, nt) in _chunk_plan(b)]

_CACHE = {}


def _build(loop_R=None):
    # loop_R: if set, wrap the pipeline in a hardware loop repeated
    # loop_R times (timing harness only; output unchanged).
    encdt = F32R if USE_F32R else F32
    nc = bacc.Bacc("TRN2", target_bir_lowering=False, debug=False)

    enc = nc.dram_tensor("enc", [BPC, S, D], F32, kind="ExternalInput")
    # zt[p, b*8+c] = z[b, p*8+c]   (z = decoder_hidden[-1] core slice)
    zt = nc.dram_tensor("zt", [128, BPC * 8], F32, kind="ExternalInput")
    w1rep = nc.dram_tensor("w1rep", [128, D], BF16, kind="ExternalInput")
    # w2t[p, c] = W2[p*8+c]
    w2t = nc.dram_tensor("w2t", [128, 8], F32, kind="ExternalInput")
    # bb128 = b[0]/128 replicated, so a ones-matmul partition-sum adds b[0]
    bb128 = nc.dram_tensor("bb128", [128, 1], F32, kind="ExternalInput")
    att = nc.dram_tensor("att", [BPC, D], F32, kind="ExternalOutput")

    with tile.TileContext(nc) as tc:
        with (
            tc.tile_pool(name="const", bufs=1) as cpool,
            tc.tile_pool(name="encp", bufs=4) as encp,
            tc.tile_pool(name="ttp", bufs=TTB) as ttp,
            tc.tile_pool(name="junkp", bufs=2) as junkp,
            tc.tile_pool(name="smallp", bufs=2) as smallp,
            tc.tile_pool(name="orowp", bufs=2) as orowp,
            tc.tile_pool(name="pscb", bufs=1, space="PSUM") as pscb,
            tc.tile_pool(name="psp", bufs=2, space="PSUM") as psp,
        ):
            # ---- constants (GPSIMD SWDGE ring; keeps HW rings clear) ----
            w1t = cpool.tile([128, D], BF16)
            nc.gpsimd.dma_start(w1t[:], w1rep.ap())
            ztt = cpool.tile([128, BPC * 8], F32)
            nc.gpsimd.dma_start(ztt[:], zt.ap())
            w2tt = cpool.tile([128, 8], F32)
            nc.gpsimd.dma_start(w2tt[:], w2t.ap())
            bbt = cpool.tile([128, 1], F32)
            nc.gpsimd.dma_start(bbt[:], bb128.ap())
            ones128 = cpool.tile([128, 1], F32)
            nc.vector.memset(ones128[:], 1.0)
            ones_sq = cpool.tile([128, 128], F32)
            nc.vector.memset(ones_sq[:], 1.0)

            # ---- prepass: cb[:, b] = tanh(z_b) @ W2 + b0, on all partitions
            tz = cpool.tile([128, BPC * 8], F32)
            nc.scalar.activation(tz[:], ztt[:], AF.Tanh)
            czp = cpool.tile([128, BPC], F32)
            zjunk = cpool.tile([128, 8], F32)
            for bi in range(BPC):
                nc.vector.scalar_tensor_tensor(
                    out=zjunk[:],
                    in0=tz[:, bi * 8 : (bi + 1) * 8],
                    scalar=1.0,
                    in1=w2tt[:],
                    op0=ALU.mult,
                    op1=ALU.mult,
                    accum_out=czp[:, bi : bi + 1],
                )
            czp2 = cpool.tile([128, BPC], F32)
            nc.vector.tensor_scalar_add(czp2[:], czp[:], bbt[:, 0:1])
            cb_ps = pscb.tile([128, BPC], F32)
            nc.tensor.matmul(cb_ps[:], ones_sq[:], czp2[:], start=True, stop=True)
            cb = cpool.tile([128, BPC], F32)
            nc.scalar.copy(cb[:], cb_ps[:])

            # ---- chunk-pipelined stream over all batches ----
            state = {}  # per-batch tiles live across the pipeline stages

            CH = _chunks()
            NTOT = len(CH)

            def load_chunk(k):
                """DMA chunk k, tanh it, fused mult+reduce scores, relu."""
                b, t0, ntl = CH[k]
                if t0 == 0:
                    st = {}
                    st["encT"] = encp.tile(
                        [128, NT * D], encdt, tag="enc", name="encT"
                    )
                    src = enc.ap()[b].rearrange("(t p) d -> p t d", p=128)
                    if USE_F32R:
                        src = src.bitcast(F32R)
                    st["src"] = src
                    st["sc"] = smallp.tile([128, NT], F32, tag="sc", name="sc")
                    st["sr"] = smallp.tile([128, NT], F32, tag="sr", name="sr")
                    st["al"] = smallp.tile([128, NT], encdt, tag="al", name="al")
                    st["ap0"] = psp.tile([1, 512], F32, tag="ap0", name="ap0")
                    st["ap1"] = psp.tile([1, 512], F32, tag="ap1", name="ap1")
                    state[b] = st
                st = state[b]
                dma_eng = nc.scalar if (DUALRING and k % 2) else nc.sync
                dma_eng.dma_start(
                    st["encT"][:, t0 * D : (t0 + ntl) * D].rearrange(
                        "p (t d) -> p t d", t=ntl
                    ),
                    st["src"][:, t0 : t0 + ntl, :],
                )
                tt = ttp.tile([128, 4 * D if CHUNK4 else 2 * D], BF16, tag="tt", name="tt")
                tin = st["encT"][:, t0 * D : (t0 + ntl) * D]
                if USE_F32R:
                    tin = tin.bitcast(F32)
                nc.scalar.activation(tt[:, 0 : ntl * D], tin, AF.Tanh)
                for kk in range(ntl):
                    t = t0 + kk
                    junk = junkp.tile([128, D], BF16 if JB16 else F32, tag="junk", name="junk")
                    # fused multiply+row-sum: out=(tt*1)*w1, accum=sum
                    # (tensor_tensor_reduce crashes the exec unit on this
                    # runtime; scalar_tensor_tensor accum works)
                    nc.vector.scalar_tensor_tensor(
                        out=junk[:],
                        in0=tt[:, kk * D : (kk + 1) * D],
                        scalar=1.0,
                        in1=w1t[:],
                        op0=ALU.mult,
                        op1=ALU.mult,
                        accum_out=st["sc"][:, t : t + 1],
                    )
                cols = slice(t0, t0 + ntl)
                # relu(score + cb) in one DVE op
                nc.vector.tensor_scalar(
                    out=st["sr"][:, cols],
                    in0=st["sc"][:, cols],
                    scalar1=cb[:, b : b + 1],
                    scalar2=0.0,
                    op0=ALU.add,
                    op1=ALU.max,
                )

            def exp_mm_chunk(k):
                """exp chunk k's scores, accumulate its weighted sum on PE."""
                b, t0, ntl = CH[k]
                st = state[b]
                cols = slice(t0, t0 + ntl)
                nc.scalar.activation(st["al"][:, cols], st["sr"][:, cols], AF.Exp)
                for kk in range(ntl):
                    t = t0 + kk
                    nc.tensor.matmul(
                        st["ap0"][:],
                        st["al"][:, t : t + 1],
                        st["encT"][:, t * D : t * D + 512],
                        start=(t == 0),
                        stop=(t == NT - 1),
                    )
                    nc.tensor.matmul(
                        st["ap1"][:],
                        st["al"][:, t : t + 1],
                        st["encT"][:, t * D + 512 : (t + 1) * D],
                        start=(t == 0),
                        stop=(t == NT - 1),
                    )

            def epi1(b):
                """softmax denominator: per-tile partition sums on the PE,
                then a tiny DVE reduce + reciprocal."""
                st = state[b]
                tot_ps = psp.tile([1, NT], F32, tag="tot", name="tot_ps")
                al_mv = st["al"][:]
                ones_st = ones128[:].bitcast(F32R) if USE_F32R else ones128[:]
                nc.tensor.matmul(tot_ps[:], ones_st, al_mv, start=True, stop=True)
                tot = smallp.tile([1, 1], F32, tag="tots", name="tot")
                nc.vector.tensor_reduce(
                    out=tot[:], in_=tot_ps[:], axis=mybir.AxisListType.X, op=ALU.add
                )
                recip = smallp.tile([1, 1], F32, tag="recip", name="recip")
                nc.vector.reciprocal(recip[:], tot[:])
                st["recip"] = recip

            def epi2(b):
                """scale the accumulated row by 1/sum (ACT and DVE halves in
                parallel) and store via the ACT HWDGE ring."""
                st = state.pop(b)
                orow0 = orowp.tile([1, 512], F32, tag="orow0", name="orow0")
                orow1 = orowp.tile([1, 512], F32, tag="orow1", name="orow1")
                nc.scalar.activation(
                    orow0[:], st["ap0"][:], AF.Copy, scale=st["recip"][0:1, 0:1]
                )
                if EPI_ACT:
                    nc.scalar.activation(
                        orow1[:], st["ap1"][:], AF.Copy,
                        scale=st["recip"][0:1, 0:1],
                    )
                else:
                    nc.vector.tensor_scalar(
                        out=orow1[:],
                        in0=st["ap1"][:],
                        scalar1=st["recip"][0:1, 0:1],
                        scalar2=None,
                        op0=ALU.mult,
                    )
                out_eng = nc.gpsimd if DUALRING else nc.scalar
                out_eng.dma_start(att.ap()[b : b + 1, 0:512], orow0[:])
                out_eng.dma_start(att.ap()[b : b + 1, 512:D], orow1[:])

            def body():
                # flat schedule with 1-chunk lookahead: exp(k) is emitted
                # after tanh(k+1) so ACT never waits on DVE's scores.
                LA = LOOKA
                for k in range(NTOT + 1 + LA):
                    if k < NTOT:
                        load_chunk(k)
                    if LA <= k <= NTOT - 1 + LA:
                        kk = k - LA
                        exp_mm_chunk(kk)
                        b, t0, ntl = CH[kk]
                        if t0 + ntl == NT:
                            epi1(b)
                    if LA + 1 <= k <= NTOT + LA:
                        kk = k - 1 - LA
                        b, t0, ntl = CH[kk]
                        if t0 + ntl == NT:
                            epi2(b)

            if loop_R is None:
                body()
            else:
                assert loop_R % BODYX == 0
                with tc.For_i(0, loop_R // BODYX, staggered_reset=STAG):
                    for _ in range(BODYX):
                        body()

    nc.compile()
    return nc


def _get_nc():
    if "nc" not in _CACHE:
        _CACHE["nc"] = _build()
    return _CACHE["nc"]


def _make_in_maps(encoder_outputs, decoder_hidden, W, b):
    enc = np.ascontiguousarray(np.asarray(encoder_outputs, dtype=np.float32))
    z = np.asarray(decoder_hidden, dtype=np.float32)[-1]  # [B, D]
    W = np.asarray(W, dtype=np.float32)
    b = np.asarray(b, dtype=np.float32)

    W1 = W[:D, 0]
    W2 = W[D:, 0]
    import ml_dtypes
    w1rep = np.ascontiguousarray(
        np.broadcast_to(W1[None, :], (128, D)).astype(ml_dtypes.bfloat16)
    )
    w2t = np.ascontiguousarray(W2.reshape(128, 8))
    bb128 = np.full((128, 1), float(b[0]) / 128.0, dtype=np.float32)

    in_maps = []
    for c in range(NCORES):
        zi = z[c * BPC : (c + 1) * BPC]  # [BPC, D]
        ztc = np.ascontiguousarray(
            zi.reshape(BPC, 128, 8).transpose(1, 0, 2).reshape(128, BPC * 8)
        )
        in_maps.append(
            {
                "enc": np.ascontiguousarray(enc[c * BPC : (c + 1) * BPC]),
                "zt": ztc,
                "w1rep": w1rep,
                "w2t": w2t,
                "bb128": bb128,
            }
        )
    return in_maps


def kernel(encoder_outputs, decoder_hidden, W, b, **_):
    in_maps = _make_in_maps(encoder_outputs, decoder_hidden, W, b)
    nc = _get_nc()
    res = run_bass_kernel_spmd(nc, in_maps, list(range(NCORES)))
    out = np.concatenate([res.results[c]["att"] for c in range(NCORES)], axis=0)
    return out.astype(np.float32)
